# revision 22
# baseline (speedup 1.0000x reference)
"""Trainium2 Bass kernel for the 32-iteration 3x3 survival automaton.

Problem: x is a 4096x4096 binary fp32 grid. 32 iterations of:
    keep cell iff its 8-neighbor live count > 3  (zero 'SAME' padding)
Output: scalar sum(x) - sum(y_final).

Strategy (8 NeuronCores, SPMD, zero inter-core communication):
  - Row-shard: core c owns rows [512c, 512c+512) and loads them plus a
    32-row halo per side; the halo is consumed one row per iteration, so
    after 32 iterations the owned rows are exact with no core-to-core
    traffic. One guard row/col of zeros emulates the 'SAME' zero padding
    (dead cells stay dead, so guards self-maintain).
  - Per-core slab: 578 rows x 4098 cols bf16, five 128-partition row tiles
    (stride 120, 8-row overlap; seam rows refreshed by DMAs every KSH
    iterations).
  - Work is balanced across ALL FOUR compute engines per iteration:
      TensorE: vertical 3-tap conv as tridiagonal matmuls. Columns are
        processed in 2048-wide pairs (4 PSUM banks). "A" pairs use two
        accumulated streams, Tri@B + (Tri+16I)@y, so one ScalarE sigmoid
        both thresholds and keeps y binary (s = N8 + 17y vs 20.5). "S"
        pairs use a single stream Tri@Hy (Hy = l+c+r) -- their threshold
        is a fused VectorE scalar_tensor_tensor (s > 4.5)*y, saving the
        second PE stream.
      ScalarE: sigmoid thresholds for A pairs (saturates to exact 1.0 /
        ~1e-26), batched 2048 wide, 8 pairs/iter.
      VectorE: B = l+r adds for tiles 0-1 (+ half of 4), Hy = B+y adds
        for S pairs, and the fused STT thresholds for S pairs.
      GpSimdE: B = l+r adds for tiles 2-3 (+ half of 4) -- otherwise idle.
  - TensorE stationaries: tri (all streams) and m16 = tri+16I (A second
    stream). Group emission order alternates per tile so dedup merges
    back-to-back reloads of the same stationary.
  - Final reduction: accum_out on the last iteration's thresholds gives
    per-partition row sums per pair; masked ones-vector matmuls reduce
    to one scalar per core. Host sums 8 partials, subtracts from sum(x).
"""

import sys

if '/opt/trn_rl_repo' not in sys.path:
    sys.path.insert(0, '/opt/trn_rl_repo')

from contextlib import ExitStack, contextmanager

import ml_dtypes
import numpy as np

import concourse.bass as bass
import concourse.tile as tile
from concourse import bacc, mybir
from concourse.bass_utils import run_bass_kernel_spmd

# ---------------------------------------------------------------- geometry
H = W = 4096
NCORES = 8
OWN = H // NCORES            # 512 rows owned per core
HALO = 32                    # rows of redundant compute per side
SLAB_R = OWN + 2 * HALO + 2  # 578 (incl. 1 guard row each side)
SLAB_C = W + 2               # 4098 (incl. 1 guard col each side)
NT = 5                       # SBUF row-tiles per slab
KSH = 7                      # seam shrink depth: refresh every KSH iters
STRIDE = 128 - 2 * KSH       # 120 (8-row overlap between tiles)
OFF = [t * STRIDE for t in range(NT)]              # 0,120,240,360,480
RT = [min(128, SLAB_R - o) for o in OFF]           # 128,128,128,128,98
MMW = 512                    # matmul output free size (1 PSUM bank)
PSW = 1024                   # threshold granularity: 2 PSUM banks
NPS = W // PSW               # 4 psum units per row-tile
MPU = PSW // MMW             # matmuls per unit per stationary (2)

# Per-tile count of psum units thresholded by ScalarE sigmoid (the rest
# go to VectorE is_gt). 16/4 keeps both ACT (~16.9us/iter) and DVE
# (~16.0) just under the TensorE bottleneck (~17.7) so PE never waits.
ACT_UNITS = [4, 3, 3, 3, 3]

F32 = mybir.dt.float32
BF16 = mybir.dt.bfloat16


@contextmanager
def _no_ldweights():
    """Emit InstMatmult with ldweights=False: reuse the PE array's currently
    loaded stationary instead of reloading per matmul."""
    orig = mybir.InstMatmult

    def mk(*a, **kw):
        kw['ldweights'] = False
        return orig(*a, **kw)

    mybir.InstMatmult = mk
    try:
        yield
    finally:
        mybir.InstMatmult = orig


def _ldw_sig(inst):
    """Signature of the stationary operand an InstLdweights loads."""
    ap = inst.ins[0]
    return (getattr(ap, 'memref', None), getattr(ap, 'offset', None),
            str(getattr(ap, 'ap', None)), str(inst.tile_position),
            str(inst.tile_size), str(getattr(inst, 'perf_mode', None)),
            str(getattr(inst, 'is_transpose', None)))


def _dedup_ldweights(nc):
    """Remove InstLdweights that reload the stationary already in the PE
    array (same weights AP, only non-loading Matmults in between). Waits on
    a removed load are pushed onto the next PE instruction; loads carrying
    semaphore updates are kept."""
    removed = 0
    for f in nc.m.functions:
        for blk in f.blocks:
            cur = None
            out = []
            pending_waits = []
            for inst in blk.instructions:
                if isinstance(inst, mybir.InstLdweights):
                    sig = _ldw_sig(inst)
                    si = inst.sync_info
                    has_upd = si is not None and len(si.on_update) > 0
                    if sig == cur and not has_upd:
                        if si is not None and len(si.on_wait) > 0:
                            pending_waits.extend(si.on_wait)
                        removed += 1
                        continue
                    cur = sig
                elif isinstance(inst, mybir.InstMatmult):
                    if inst.is_transpose or getattr(inst, 'ldweights', None) is not False:
                        cur = None
                elif type(inst).__name__ == 'InstMatmultMx':
                    cur = None
                if pending_waits and isinstance(
                        inst, (mybir.InstLdweights, mybir.InstMatmult)):
                    si = inst.sync_info
                    if si is None:
                        inst.sync_info = mybir.SyncInfo(
                            on_wait=list(pending_waits), on_update=[])
                    else:
                        si.on_wait = list(si.on_wait) + pending_waits
                    pending_waits = []
                out.append(inst)
            assert not pending_waits
            if len(out) != len(blk.instructions):
                blk.instructions[:] = out
    return removed


def _build(iters: int):
    nc = bacc.Bacc("TRN2", target_bir_lowering=False, debug=False)
    x_d = nc.dram_tensor("x", [SLAB_R, SLAB_C], BF16, kind="ExternalInput").ap()
    tri_d = nc.dram_tensor("tri", [128, 128], BF16, kind="ExternalInput").ap()
    m16_d = nc.dram_tensor("m16", [128, 128], BF16, kind="ExternalInput").ap()
    rmask_d = nc.dram_tensor("rmask", [NT, 128], F32, kind="ExternalInput").ap()
    out_d = nc.dram_tensor("ysum", [1, 1], F32, kind="ExternalOutput").ap()

    add = mybir.AluOpType.add

    with tile.TileContext(nc) as tc, ExitStack() as ctx:
        const_pool = ctx.enter_context(tc.tile_pool(name="const", bufs=1))
        ypool = ctx.enter_context(tc.tile_pool(name="y", bufs=1))
        bpool = ctx.enter_context(tc.tile_pool(name="b", bufs=1))

        tri_sb = const_pool.tile([128, 128], BF16, tag="tri")
        nc.sync.dma_start(tri_sb[:], tri_d[:])
        m16_sb = const_pool.tile([128, 128], BF16, tag="m16")
        nc.sync.dma_start(m16_sb[:], m16_d[:])
        rmask_sb = []
        for t in range(NT):
            rm = const_pool.tile([128, 1], F32, tag=f"rmask{t}", name=f"rmask{t}")
            nc.sync.dma_start(rm[:], rmask_d[t:t + 1, :])
            rmask_sb.append(rm)
        bias_sb = const_pool.tile([128, 1], F32, tag="biasc", name="biasc")
        nc.gpsimd.memset(bias_sb[:], -2460.0)

        y_sb = [ypool.tile([RT[t], SLAB_C], BF16, tag=f"y{t}", name=f"y{t}")
                for t in range(NT)]
        b_sb = [bpool.tile([RT[t], W], BF16, tag=f"b{t}", name=f"b{t}")
                for t in range(NT)]

        # load (host already converted to bf16)
        for t in range(NT):
            nc.sync.dma_start(y_sb[t][:], x_d[OFF[t]:OFF[t] + RT[t], :])

        def emit_adds(t):
            nc.vector.tensor_tensor(
                b_sb[t][:], y_sb[t][:, 0:W], y_sb[t][:, 2:W + 2], op=add)

        def emit_seam(t):
            # refresh the 2*KSH-row overlap between tiles t and t+1 (each
            # tile's outer KSH rows go stale over KSH iterations)
            nc.sync.dma_start(y_sb[t][128 - KSH:128, :],
                              y_sb[t + 1][KSH:2 * KSH, :])
            nc.sync.dma_start(y_sb[t + 1][0:KSH, :],
                              y_sb[t][STRIDE:STRIDE + KSH, :])

        acc_list = []  # (tile, acc_tile) pairs written on the last iteration

        def mm(first, *args, **kw):
            if first:
                nc.tensor.matmul(*args, **kw)
            else:
                with _no_ldweights():
                    nc.tensor.matmul(*args, **kw)

        def emit_mms_thresholds(psum_pool, it, t, accum=False):
            r = RT[t]
            psums = [psum_pool.tile([r, PSW], F32, tag="ps",
                                    name=f"ps_{it}_{t}_{u}")
                     for u in range(NPS)]

            def group(w_sb, units, first, g_start):
                is_tri = w_sb is tri_sb
                for u in units:
                    for h in range(MPU):
                        c0 = u * PSW + h * MMW
                        src = (b_sb[t][0:r, c0:c0 + MMW] if is_tri
                               else y_sb[t][:, 1 + c0:1 + c0 + MMW])
                        mm(first, psums[u][:, h * MMW:(h + 1) * MMW],
                           w_sb[0:r, 0:r], src,
                           start=g_start, stop=not g_start)
                        first = False

            # Full-tile stationary groups with tile-parity alternation so
            # adjacent tiles end / begin with the same stationary and the
            # dedup pass removes the reload. With 4 rotating psum units a
            # unit's threshold still starts mid-second-group, so the slot
            # needed by the next tile frees in time.
            if t % 2 == 0:
                group(tri_sb, (0, 1, 2, 3), True, True)
                group(m16_sb, (0, 1, 2, 3), True, False)
            else:
                group(m16_sb, (0, 1, 2, 3), True, True)
                group(tri_sb, (0, 1, 2, 3), True, False)

            def acc_for(kind):
                if not accum:
                    return None
                a = const_pool.tile([128, 1], F32, tag=f"acc{t}_{kind}",
                                    name=f"acc{t}_{kind}")
                acc_list.append((t, a))
                return a[0:r, 0:1]

            for u in range(NPS):
                dst = y_sb[t][:, 1 + u * PSW:1 + (u + 1) * PSW]
                aout = acc_for(u)
                if u < ACT_UNITS[t]:
                    nc.scalar.activation(
                        dst, psums[u][:],
                        mybir.ActivationFunctionType.Sigmoid,
                        bias=bias_sb[0:r, 0:1], scale=120.0,
                        accum_out=aout)
                else:
                    if accum:
                        nc.vector.tensor_scalar(
                            dst, psums[u][:], 20.5, 0.0,
                            op0=mybir.AluOpType.is_gt,
                            op1=mybir.AluOpType.add, accum_out=aout)
                    else:
                        nc.vector.tensor_scalar(
                            dst, psums[u][:], 20.5, None,
                            op0=mybir.AluOpType.is_gt)

        # Software-pipelined wavefront with seam shrinkage: tiles overlap by
        # 2*KSH rows, so seams need refreshing only every KSH-th iteration.
        # On non-refresh boundaries a tile's next-iteration adds depend only
        # on its own thresholds and are emitted right after it -- TensorE
        # rolls across the iteration boundary with no bubble. On refresh
        # boundaries, seams are refreshed as soon as both neighbor tiles are
        # thresholded.
        with tc.tile_pool(name="ps", bufs=4, space="PSUM") as psum_pool:
            for t in range(NT):
                emit_adds(t)
            for it in range(iters):
                last = it == iters - 1
                refresh = (it % KSH == KSH - 1) and not last
                for t in range(NT):
                    emit_mms_thresholds(psum_pool, it, t, accum=last)
                    if last:
                        continue
                    if refresh:
                        if t >= 1:
                            emit_seam(t - 1)
                        if t >= 2:
                            emit_adds(t - 2)
                    else:
                        emit_adds(t)
                if not last and refresh:
                    emit_adds(NT - 2)
                    emit_adds(NT - 1)

        # masked dot of the per-row accumulators from the last iteration's
        # thresholds: ysum = sum_t rmask[t] . (row sums of tile t)
        with tc.tile_pool(name="sps", bufs=1, space="PSUM") as spsum_pool:
            sps = spsum_pool.tile([1, 1], F32, tag="sum", name="sps")
            n_mm = len(acc_list)
            for k, (t, a) in enumerate(acc_list):
                nc.tensor.matmul(
                    sps[:], rmask_sb[t][0:RT[t], 0:1],
                    a[0:RT[t], 0:1],
                    start=(k == 0), stop=(k == n_mm - 1))
            ssb = const_pool.tile([1, 1], F32, tag="ssum", name="ssb")
            nc.vector.tensor_copy(ssb[:], sps[:])
            nc.sync.dma_start(out_d[:], ssb[:])

    _dedup_ldweights(nc)
    # After dedup, the "most recent ldweights" a matmul's extra waits would
    # be moved to can sit many matmuls earlier in the PE stream — waiting
    # there can deadlock against producers scheduled in between. Skip the
    # pass; generate_event_semaphores enforces the 1-wait constraint by
    # splitting waits into standalone event-sem instructions in place.
    nc.move_matmul_waits_to_ldweights = lambda: None
    nc.compile()
    return nc


def _consts():
    i = np.arange(128)
    tri = (np.abs(i[:, None] - i[None, :]) <= 1).astype(np.float32)
    m16 = tri + 16.0 * np.eye(128, dtype=np.float32)
    # valid-row masks for the final sum: slab rows [33, 545) are the owned
    # 512 rows; each row is summed from the tile where it is seam-valid
    # (interior partitions after the last iteration).
    rmask = np.zeros((NT, 128), np.float32)
    # interior partitions [KSH, 128-KSH) are seam-valid; tile 0 has no
    # upper seam (slab edge) and tile 4 no lower seam
    bounds = [(33, 121), (7, 121), (7, 121), (7, 121), (7, 89)]
    for t, (a, b) in enumerate(bounds):
        rmask[t, a:b] = 1.0
    assert sum(b - a for a, b in bounds) == OWN
    bf = ml_dtypes.bfloat16
    return tri.astype(bf), m16.astype(bf), rmask


def _slabs(x: np.ndarray):
    g = np.zeros((H + 2 * HALO + 2, SLAB_C), ml_dtypes.bfloat16)
    g[HALO + 1:HALO + 1 + H, 1:1 + W] = x  # 0/1 values: exact in bf16
    return [np.ascontiguousarray(g[c * OWN:c * OWN + SLAB_R])
            for c in range(NCORES)]


_CACHE = {}


def _get_nc(iters: int):
    if iters not in _CACHE:
        _CACHE[iters] = _build(iters)
    return _CACHE[iters]


def kernel(x: np.ndarray, convs) -> np.ndarray:
    iters = int(convs)
    x = np.asarray(x, np.float32)
    assert x.shape == (H, W)
    nc = _get_nc(iters)
    tri, m16, rmask = _consts()
    in_maps = [{"x": s, "tri": tri, "m16": m16, "rmask": rmask}
               for s in _slabs(x)]
    res = run_bass_kernel_spmd(nc, in_maps, core_ids=list(range(NCORES)))
    y_sum = sum(float(res.results[c]["ysum"][0, 0]) for c in range(NCORES))
    x_sum = float(x.astype(np.float64).sum())
    return np.float32(x_sum - y_sum)


if __name__ == "__main__":
    rng = np.random.default_rng(0)
    x = np.round(rng.random((H, W))).astype(np.float32)
    got = kernel(x, 32)
    from scipy import signal
    K = np.array([[1, 1, 1], [1, 0, 1], [1, 1, 1]], np.float32)
    y = x.copy()
    for _ in range(32):
        s = signal.convolve2d(y, K, mode='same')
        y = np.where(s > 3.0, y, 0).astype(np.float32)
    want = x.sum(dtype=np.float64) - y.sum(dtype=np.float64)
    print(f"got {got}, want {want}, rel {abs(got - want) / abs(want):.3e}")


# revision 23
# speedup vs baseline: 1.0051x; 1.0051x over previous
"""Trainium2 Bass kernel for the 32-iteration 3x3 survival automaton.

Problem: x is a 4096x4096 binary fp32 grid. 32 iterations of:
    keep cell iff its 8-neighbor live count > 3  (zero 'SAME' padding)
Output: scalar sum(x) - sum(y_final).

Strategy (8 NeuronCores, SPMD, zero inter-core communication):
  - Row-shard: core c owns rows [512c, 512c+512) and loads them plus a
    32-row halo per side; the halo is consumed one row per iteration, so
    after 32 iterations the owned rows are exact with no core-to-core
    traffic. One guard row/col of zeros emulates the 'SAME' zero padding
    (dead cells stay dead, so guards self-maintain).
  - Per-core slab: 578 rows x 4098 cols bf16, five 128-partition row tiles
    (stride 120, 8-row overlap; seam rows refreshed by DMAs every KSH
    iterations).
  - Work is balanced across ALL FOUR compute engines per iteration:
      TensorE: vertical 3-tap conv as tridiagonal matmuls. Columns are
        processed in 2048-wide pairs (4 PSUM banks). "A" pairs use two
        accumulated streams, Tri@B + (Tri+16I)@y, so one ScalarE sigmoid
        both thresholds and keeps y binary (s = N8 + 17y vs 20.5). "S"
        pairs use a single stream Tri@Hy (Hy = l+c+r) -- their threshold
        is a fused VectorE scalar_tensor_tensor (s > 4.5)*y, saving the
        second PE stream.
      ScalarE: sigmoid thresholds for A pairs (saturates to exact 1.0 /
        ~1e-26), batched 2048 wide, 8 pairs/iter.
      VectorE: B = l+r adds for tiles 0-1 (+ half of 4), Hy = B+y adds
        for S pairs, and the fused STT thresholds for S pairs.
      GpSimdE: B = l+r adds for tiles 2-3 (+ half of 4) -- otherwise idle.
  - TensorE stationaries: tri (all streams) and m16 = tri+16I (A second
    stream). Group emission order alternates per tile so dedup merges
    back-to-back reloads of the same stationary.
  - Final reduction: accum_out on the last iteration's thresholds gives
    per-partition row sums per pair; masked ones-vector matmuls reduce
    to one scalar per core. Host sums 8 partials, subtracts from sum(x).
"""

import sys

if '/opt/trn_rl_repo' not in sys.path:
    sys.path.insert(0, '/opt/trn_rl_repo')

from contextlib import ExitStack, contextmanager

import ml_dtypes
import numpy as np

import concourse.bass as bass
import concourse.tile as tile
from concourse import bacc, mybir
from concourse.bass_utils import run_bass_kernel_spmd

# ---------------------------------------------------------------- geometry
H = W = 4096
NCORES = 8
OWN = H // NCORES            # 512 rows owned per core
HALO = 32                    # rows of redundant compute per side
SLAB_R = OWN + 2 * HALO + 2  # 578 (incl. 1 guard row each side)
SLAB_C = W + 2               # 4098 (incl. 1 guard col each side)
NT = 5                       # SBUF row-tiles per slab
KSH = 7                      # seam shrink depth: refresh every KSH iters
STRIDE = 128 - 2 * KSH       # 120 (8-row overlap between tiles)
OFF = [t * STRIDE for t in range(NT)]              # 0,120,240,360,480
RT = [min(128, SLAB_R - o) for o in OFF]           # 128,128,128,128,98
MMW = 512                    # matmul output free size (1 PSUM bank)
PSW = 1024                   # threshold granularity: 2 PSUM banks
NPS = W // PSW               # 4 psum units per row-tile
MPU = PSW // MMW             # matmuls per unit per stationary (2)

# Per-tile count of psum units thresholded by ScalarE sigmoid (the rest
# go to VectorE is_gt). 16/4 keeps both ACT (~16.9us/iter) and DVE
# (~16.0) just under the TensorE bottleneck (~17.7) so PE never waits.
ACT_UNITS = [4, 3, 3, 3, 3]

F32 = mybir.dt.float32
BF16 = mybir.dt.bfloat16


@contextmanager
def _no_ldweights():
    """Emit InstMatmult with ldweights=False: reuse the PE array's currently
    loaded stationary instead of reloading per matmul."""
    orig = mybir.InstMatmult

    def mk(*a, **kw):
        kw['ldweights'] = False
        return orig(*a, **kw)

    mybir.InstMatmult = mk
    try:
        yield
    finally:
        mybir.InstMatmult = orig


def _ldw_sig(inst):
    """Signature of the stationary operand an InstLdweights loads."""
    ap = inst.ins[0]
    return (getattr(ap, 'memref', None), getattr(ap, 'offset', None),
            str(getattr(ap, 'ap', None)), str(inst.tile_position),
            str(inst.tile_size), str(getattr(inst, 'perf_mode', None)),
            str(getattr(inst, 'is_transpose', None)))


def _dedup_ldweights(nc):
    """Remove InstLdweights that reload the stationary already in the PE
    array (same weights AP, only non-loading Matmults in between). Waits on
    a removed load are pushed onto the next PE instruction; loads carrying
    semaphore updates are kept."""
    removed = 0
    for f in nc.m.functions:
        for blk in f.blocks:
            cur = None
            out = []
            pending_waits = []
            for inst in blk.instructions:
                if isinstance(inst, mybir.InstLdweights):
                    sig = _ldw_sig(inst)
                    si = inst.sync_info
                    has_upd = si is not None and len(si.on_update) > 0
                    if sig == cur and not has_upd:
                        if si is not None and len(si.on_wait) > 0:
                            pending_waits.extend(si.on_wait)
                        removed += 1
                        continue
                    cur = sig
                elif isinstance(inst, mybir.InstMatmult):
                    if inst.is_transpose or getattr(inst, 'ldweights', None) is not False:
                        cur = None
                elif type(inst).__name__ == 'InstMatmultMx':
                    cur = None
                if pending_waits and isinstance(
                        inst, (mybir.InstLdweights, mybir.InstMatmult)):
                    si = inst.sync_info
                    if si is None:
                        inst.sync_info = mybir.SyncInfo(
                            on_wait=list(pending_waits), on_update=[])
                    else:
                        si.on_wait = list(si.on_wait) + pending_waits
                    pending_waits = []
                out.append(inst)
            assert not pending_waits
            if len(out) != len(blk.instructions):
                blk.instructions[:] = out
    return removed


def _build(iters: int):
    nc = bacc.Bacc("TRN2", target_bir_lowering=False, debug=False)
    x_d = nc.dram_tensor("x", [SLAB_R, SLAB_C], BF16, kind="ExternalInput").ap()
    tri_d = nc.dram_tensor("tri", [128, 128], BF16, kind="ExternalInput").ap()
    m16_d = nc.dram_tensor("m16", [128, 128], BF16, kind="ExternalInput").ap()
    rmask_d = nc.dram_tensor("rmask", [NT, 128], F32, kind="ExternalInput").ap()
    out_d = nc.dram_tensor("ysum", [1, 1], F32, kind="ExternalOutput").ap()

    add = mybir.AluOpType.add

    with tile.TileContext(nc) as tc, ExitStack() as ctx:
        const_pool = ctx.enter_context(tc.tile_pool(name="const", bufs=1))
        ypool = ctx.enter_context(tc.tile_pool(name="y", bufs=1))
        bpool = ctx.enter_context(tc.tile_pool(name="b", bufs=1))

        tri_sb = const_pool.tile([128, 128], BF16, tag="tri")
        nc.sync.dma_start(tri_sb[:], tri_d[:])
        m16_sb = const_pool.tile([128, 128], BF16, tag="m16")
        nc.sync.dma_start(m16_sb[:], m16_d[:])
        rmask_sb = []
        for t in range(NT):
            rm = const_pool.tile([128, 1], F32, tag=f"rmask{t}", name=f"rmask{t}")
            nc.sync.dma_start(rm[:], rmask_d[t:t + 1, :])
            rmask_sb.append(rm)
        bias_sb = const_pool.tile([128, 1], F32, tag="biasc", name="biasc")
        nc.gpsimd.memset(bias_sb[:], -2460.0)

        y_sb = [ypool.tile([RT[t], SLAB_C], BF16, tag=f"y{t}", name=f"y{t}")
                for t in range(NT)]
        b_sb = [bpool.tile([RT[t], W], BF16, tag=f"b{t}", name=f"b{t}")
                for t in range(NT)]

        # load (host already converted to bf16)
        for t in range(NT):
            nc.sync.dma_start(y_sb[t][:], x_d[OFF[t]:OFF[t] + RT[t], :])

        def emit_adds(t):
            nc.vector.tensor_tensor(
                b_sb[t][:], y_sb[t][:, 0:W], y_sb[t][:, 2:W + 2], op=add)

        def emit_seam(t):
            # refresh the 2*KSH-row overlap between tiles t and t+1 (each
            # tile's outer KSH rows go stale over KSH iterations)
            nc.sync.dma_start(y_sb[t][128 - KSH:128, :],
                              y_sb[t + 1][KSH:2 * KSH, :])
            nc.sync.dma_start(y_sb[t + 1][0:KSH, :],
                              y_sb[t][STRIDE:STRIDE + KSH, :])

        acc_list = []  # (tile, acc_tile) pairs written on the last iteration

        def mm(first, *args, **kw):
            if first:
                nc.tensor.matmul(*args, **kw)
            else:
                with _no_ldweights():
                    nc.tensor.matmul(*args, **kw)

        def emit_mms_thresholds(psum_pool, it, t, accum=False):
            r = RT[t]
            psums = [psum_pool.tile([r, PSW], F32, tag="ps",
                                    name=f"ps_{it}_{t}_{u}")
                     for u in range(NPS)]

            def group(w_sb, units, first, g_start):
                is_tri = w_sb is tri_sb
                for u in units:
                    for h in range(MPU):
                        c0 = u * PSW + h * MMW
                        src = (b_sb[t][0:r, c0:c0 + MMW] if is_tri
                               else y_sb[t][:, 1 + c0:1 + c0 + MMW])
                        mm(first, psums[u][:, h * MMW:(h + 1) * MMW],
                           w_sb[0:r, 0:r], src,
                           start=g_start, stop=not g_start)
                        first = False

            # Half-tile stationary groups, ordered [u01: tri,m16]
            # [u23: m16,tri]: unit-0's sigmoid can start mid-tile (fine
            # PSUM slot rotation) while group boundaries still merge
            # LDWEIGHTS (u01 ends m16 / u23 begins m16; u23 ends tri /
            # next tile begins tri -- dedup removes the reloads).
            group(tri_sb, (0, 1), True, True)
            group(m16_sb, (0, 1), True, False)
            group(m16_sb, (2, 3), True, True)
            group(tri_sb, (2, 3), True, False)

            def acc_for(kind):
                if not accum:
                    return None
                a = const_pool.tile([128, 1], F32, tag=f"acc{t}_{kind}",
                                    name=f"acc{t}_{kind}")
                acc_list.append((t, a))
                return a[0:r, 0:1]

            for u in range(NPS):
                dst = y_sb[t][:, 1 + u * PSW:1 + (u + 1) * PSW]
                aout = acc_for(u)
                if u < ACT_UNITS[t]:
                    nc.scalar.activation(
                        dst, psums[u][:],
                        mybir.ActivationFunctionType.Sigmoid,
                        bias=bias_sb[0:r, 0:1], scale=120.0,
                        accum_out=aout)
                else:
                    if accum:
                        nc.vector.tensor_scalar(
                            dst, psums[u][:], 20.5, 0.0,
                            op0=mybir.AluOpType.is_gt,
                            op1=mybir.AluOpType.add, accum_out=aout)
                    else:
                        nc.vector.tensor_scalar(
                            dst, psums[u][:], 20.5, None,
                            op0=mybir.AluOpType.is_gt)

        # Software-pipelined wavefront with seam shrinkage: tiles overlap by
        # 2*KSH rows, so seams need refreshing only every KSH-th iteration.
        # On non-refresh boundaries a tile's next-iteration adds depend only
        # on its own thresholds and are emitted right after it -- TensorE
        # rolls across the iteration boundary with no bubble. On refresh
        # boundaries, seams are refreshed as soon as both neighbor tiles are
        # thresholded.
        with tc.tile_pool(name="ps", bufs=4, space="PSUM") as psum_pool:
            for t in range(NT):
                emit_adds(t)
            for it in range(iters):
                last = it == iters - 1
                refresh = (it % KSH == KSH - 1) and not last
                for t in range(NT):
                    emit_mms_thresholds(psum_pool, it, t, accum=last)
                    if last:
                        continue
                    if refresh:
                        if t >= 1:
                            emit_seam(t - 1)
                        if t >= 2:
                            emit_adds(t - 2)
                    else:
                        emit_adds(t)
                if not last and refresh:
                    emit_adds(NT - 2)
                    emit_adds(NT - 1)

        # masked dot of the per-row accumulators from the last iteration's
        # thresholds: ysum = sum_t rmask[t] . (row sums of tile t)
        with tc.tile_pool(name="sps", bufs=1, space="PSUM") as spsum_pool:
            sps = spsum_pool.tile([1, 1], F32, tag="sum", name="sps")
            n_mm = len(acc_list)
            for k, (t, a) in enumerate(acc_list):
                nc.tensor.matmul(
                    sps[:], rmask_sb[t][0:RT[t], 0:1],
                    a[0:RT[t], 0:1],
                    start=(k == 0), stop=(k == n_mm - 1))
            ssb = const_pool.tile([1, 1], F32, tag="ssum", name="ssb")
            nc.vector.tensor_copy(ssb[:], sps[:])
            nc.sync.dma_start(out_d[:], ssb[:])

    _dedup_ldweights(nc)
    # After dedup, the "most recent ldweights" a matmul's extra waits would
    # be moved to can sit many matmuls earlier in the PE stream — waiting
    # there can deadlock against producers scheduled in between. Skip the
    # pass; generate_event_semaphores enforces the 1-wait constraint by
    # splitting waits into standalone event-sem instructions in place.
    nc.move_matmul_waits_to_ldweights = lambda: None
    nc.compile()
    return nc


def _consts():
    i = np.arange(128)
    tri = (np.abs(i[:, None] - i[None, :]) <= 1).astype(np.float32)
    m16 = tri + 16.0 * np.eye(128, dtype=np.float32)
    # valid-row masks for the final sum: slab rows [33, 545) are the owned
    # 512 rows; each row is summed from the tile where it is seam-valid
    # (interior partitions after the last iteration).
    rmask = np.zeros((NT, 128), np.float32)
    # interior partitions [KSH, 128-KSH) are seam-valid; tile 0 has no
    # upper seam (slab edge) and tile 4 no lower seam
    bounds = [(33, 121), (7, 121), (7, 121), (7, 121), (7, 89)]
    for t, (a, b) in enumerate(bounds):
        rmask[t, a:b] = 1.0
    assert sum(b - a for a, b in bounds) == OWN
    bf = ml_dtypes.bfloat16
    return tri.astype(bf), m16.astype(bf), rmask


def _slabs(x: np.ndarray):
    g = np.zeros((H + 2 * HALO + 2, SLAB_C), ml_dtypes.bfloat16)
    g[HALO + 1:HALO + 1 + H, 1:1 + W] = x  # 0/1 values: exact in bf16
    return [np.ascontiguousarray(g[c * OWN:c * OWN + SLAB_R])
            for c in range(NCORES)]


_CACHE = {}


def _get_nc(iters: int):
    if iters not in _CACHE:
        _CACHE[iters] = _build(iters)
    return _CACHE[iters]


def kernel(x: np.ndarray, convs) -> np.ndarray:
    iters = int(convs)
    x = np.asarray(x, np.float32)
    assert x.shape == (H, W)
    nc = _get_nc(iters)
    tri, m16, rmask = _consts()
    in_maps = [{"x": s, "tri": tri, "m16": m16, "rmask": rmask}
               for s in _slabs(x)]
    res = run_bass_kernel_spmd(nc, in_maps, core_ids=list(range(NCORES)))
    y_sum = sum(float(res.results[c]["ysum"][0, 0]) for c in range(NCORES))
    x_sum = float(x.astype(np.float64).sum())
    return np.float32(x_sum - y_sum)


if __name__ == "__main__":
    rng = np.random.default_rng(0)
    x = np.round(rng.random((H, W))).astype(np.float32)
    got = kernel(x, 32)
    from scipy import signal
    K = np.array([[1, 1, 1], [1, 0, 1], [1, 1, 1]], np.float32)
    y = x.copy()
    for _ in range(32):
        s = signal.convolve2d(y, K, mode='same')
        y = np.where(s > 3.0, y, 0).astype(np.float32)
    want = x.sum(dtype=np.float64) - y.sum(dtype=np.float64)
    print(f"got {got}, want {want}, rel {abs(got - want) / abs(want):.3e}")
